# revision 16
# baseline (speedup 1.0000x reference)
"""Causal GQA self-attention with RoPE for TRN2, 8 NeuronCores.

Problem: B=2, S=2048, D=2048, H=16 q-heads, KV=4 kv-heads, HD=128.

Sharding: core c = (batch b = c//4, kv-group g = c%4). Each core computes
q-heads 4g..4g+3 and kv-head g for batch b in the transposed (S^T) layout,
then a partial output projection; host sums the 4 partials per batch.

v4 notes (PE is the bottleneck; keep it saturated end to end):
  - weights pre-packed on host into SBUF partition-major layout (4-16KB
    DMA descriptors); DMA dispatch costs ~650ns each on a queue, so
    loads are dual-dispatched from the sync AND gpsimd queues, ordered
    by need (wk, wv, x chunks, ...). K/V projections run kd-outer,
    chasing the x stream.
  - RoPE rotate-half is a PE permutation matmul per [128,512] chunk
    (engines cannot cross partitions; the old SBUF->SBUF swap DMA had
    ~7us latency and stalled attention start), then 3 narrow DVE ops.
  - softmax denominator via DVE accumulation of exp tiles + one
    ones-matmul colsum pair per (head, query block).
  - causal masking: only the 128-wide diagonal sliver of each diagonal
    key chunk is mask-multiplied (every diagonal chunk uses the same
    triangular [128,128] mask); QK/PV matmuls and the DVE accumulation
    are narrowed to the live query range.
  - attention runs jq-outer / h-inner; output-projection matmuls are
    drained into the exp-latency gaps of the attention stream.
  - output stored f16; host sums partials in f32.
"""
import sys

sys.path.insert(0, "/opt/trn_rl_repo")

import numpy as np

import concourse.tile as tile
from concourse import bacc, mybir
from concourse.bass_utils import run_bass_kernel_spmd

F32 = mybir.dt.float32
F16 = mybir.dt.float16
AF = mybir.ActivationFunctionType
OP = mybir.AluOpType

P = 128          # partitions / head dim
S = 2048         # sequence length
D = 2048         # model dim
NH = 4           # q heads per core
QW = NH * P      # q projection width per core (512)
NKD = D // P     # contraction chunks (16)
QCH = 512        # query chunk (free dim of attention matmuls)
NQC = S // QCH   # 4
KCH = P          # key chunk (128, on partitions)
NKC = S // KCH   # 16
SCALE = float(P) ** -0.5


def _host_constants():
    inv = 1.0 / (10000.0 ** (np.arange(0, P, 2, dtype=np.float64) / P))  # [64]
    pos = np.arange(S, dtype=np.float64)
    freqs = pos[:, None] * inv[None, :]                  # [S, 64]
    emb = np.concatenate([freqs, freqs], axis=-1)        # [S, 128]
    cosT = np.cos(emb).T.astype(np.float16).copy()       # [128, S]
    sinT = np.sin(emb).T.astype(np.float16)
    sinT[: P // 2] *= np.float16(-1.0)                   # fold rotate_half sign
    sinT = sinT.copy()
    # triangular mask for the 128-wide diagonal sliver: m[p, q] = q >= p
    tri = (np.arange(P)[None, :] >= np.arange(P)[:, None]).astype(np.float16)
    # full step masks for the jq0/h0 full-width path: mask[p,j,q] = q >= p+128j
    q = np.arange(QCH)[None, None, :]
    p = np.arange(P)[:, None, None]
    j = np.arange(4)[None, :, None]
    masks = (q >= p + KCH * j).astype(np.float16)        # [128, 4, 512]
    ones = np.ones((P, P), dtype=np.float16)
    # rotate-half permutation: out[m] = in[(m+64) % 128]
    perm = np.zeros((P, P), dtype=np.float16)
    perm[(np.arange(P) + P // 2) % P, np.arange(P)] = 1.0
    return cosT, sinT, tri, masks, ones, perm


def build_nc():
    cosT_np, sinT_np, tri_np, masks_np, ones_np, perm_np = _host_constants()

    nc = bacc.Bacc(None)
    # weights arrive pre-packed as [128, ...] partition-major arrays
    xT_d = nc.dram_tensor("xT", [D, S], F16, kind="ExternalInput")
    wq_d = nc.dram_tensor("wq", [P, NKD * QW], F16, kind="ExternalInput")
    wk_d = nc.dram_tensor("wk", [P, NKD * P], F16, kind="ExternalInput")
    wv_d = nc.dram_tensor("wv", [P, NKD * P], F16, kind="ExternalInput")
    wo_d = nc.dram_tensor("wo", [P, NH * D], F16, kind="ExternalInput")
    out_d = nc.dram_tensor("outT", [D, S], F16, kind="ExternalOutput")

    cos_d = nc.inline_tensor(cosT_np, name="cosT")
    sin_d = nc.inline_tensor(sinT_np, name="sinT")
    tri_d = nc.inline_tensor(tri_np, name="trim")
    mask_d = nc.inline_tensor(masks_np, name="masks")
    ones_d = nc.inline_tensor(ones_np, name="onesm")
    perm_d = nc.inline_tensor(perm_np, name="permm")

    xT_v = xT_d[:].rearrange("(kd p) s -> p kd s", p=P)

    # alternate DMA dispatch between the sync and gpsimd queues
    dq = [0]

    def dma(out, in_):
        eng = nc.sync if dq[0] % 2 == 0 else nc.gpsimd
        dq[0] += 1
        eng.dma_start(out, in_)

    with tile.TileContext(nc) as tc:
        with tc.tile_pool(name="persist", bufs=1) as pp:
            qT = pp.tile([P, NH, S], F16)        # q^T; attention overwrites
            kT = pp.tile([P, S], F16)
            vT = pp.tile([P, S], F16)
            vK = pp.tile([P, NKC, P], F16)       # V as (kpos, kchunk, hd)
            tri_t = pp.tile([P, P], F16)
            mask_t = pp.tile([P, 4, QCH], F16)
            ones_t = pp.tile([P, P], F16)        # all-ones for colsum
            perm_t = pp.tile([P, P], F16)

            # attention output overwrites qT in place: slice (h, jq-chunk) is
            # written only after every read of that same slice is done.
            attnT = qT

            # ======== Phase 1: QKV projections + RoPE ====================
            with tc.tile_pool(name="xp", bufs=1) as xp, \
                 tc.tile_pool(name="wp", bufs=1) as wp, \
                 tc.tile_pool(name="p1", bufs=2) as p1, \
                 tc.tile_pool(name="p1c", bufs=1) as p1c, \
                 tc.tile_pool(name="psP", bufs=1, space="PSUM") as psP:
                # dispatch order == need order: wk, wv, x chunks, the rest
                wkt = wp.tile([P, NKD, P], F16)
                wvt = wp.tile([P, NKD, P], F16)
                nc.sync.dma_start(
                    wkt[:], wk_d[:].rearrange("p (kd c) -> p kd c", c=P))
                nc.gpsimd.dma_start(
                    wvt[:], wv_d[:].rearrange("p (kd c) -> p kd c", c=P))
                xf = xp.tile([P, NKD, S], F16)
                for kd in range(NKD):
                    if kd < 2:
                        # first chunks gate the first matmuls: split for
                        # parallel queue transfer
                        dma(xf[:64, kd, :], xT_v[:64, kd, :])
                        dma(xf[64:, kd, :], xT_v[64:, kd, :])
                    else:
                        dma(xf[:, kd, :], xT_v[:, kd, :])
                cos_t = p1c.tile([P, S], F16)
                sin_t = p1c.tile([P, S], F16)
                dma(cos_t[:], cos_d[:])
                dma(sin_t[:], sin_d[:])
                dma(perm_t[:], perm_d[:])
                wqt = wp.tile([P, NKD, QW], F16)
                wq_vv = wq_d[:].rearrange("p (kd c) -> p kd c", c=QW)
                for k4 in range(4):
                    dma(wqt[:, 4 * k4 : 4 * (k4 + 1), :],
                        wq_vv[:, 4 * k4 : 4 * (k4 + 1), :])
                dma(tri_t[:], tri_d[:])
                dma(mask_t[:], mask_d[:])
                dma(ones_t[:], ones_d[:])

                def rope_chunk(dst, rot_ps, cslice):
                    # dst = dst*cos + perm(dst)*sin ; rot_ps holds perm(dst)
                    tmpv = p1.tile([P, QCH], F16, tag="ropet", bufs=3)
                    nc.vector.tensor_tensor(
                        tmpv[:], rot_ps[:], sin_t[:, cslice], OP.mult)
                    nc.vector.tensor_tensor(
                        dst, dst, cos_t[:, cslice], OP.mult)
                    nc.vector.tensor_tensor(dst, dst, tmpv[:], OP.add)

                # K and V projections, kd-outer, 8 PSUM accumulators
                psK = [psP.tile([P, QCH], F32, tag=f"pk{jr}", name=f"pk{jr}")
                       for jr in range(NQC)]
                psV = [psP.tile([P, QCH], F32, tag=f"pv{jr}", name=f"pv{jr}")
                       for jr in range(NQC)]
                for kd in range(NKD):
                    for jr in range(NQC):
                        nc.tensor.matmul(
                            psK[jr][:], wkt[:, kd, :],
                            xf[:, kd, jr * QCH : (jr + 1) * QCH],
                            start=(kd == 0), stop=(kd == NKD - 1),
                        )
                    for jr in range(NQC):
                        nc.tensor.matmul(
                            psV[jr][:], wvt[:, kd, :],
                            xf[:, kd, jr * QCH : (jr + 1) * QCH],
                            start=(kd == 0), stop=(kd == NKD - 1),
                        )
                for jr in range(NQC):
                    nc.scalar.copy(
                        out=kT[:, jr * QCH : (jr + 1) * QCH], in_=psK[jr][:]
                    )
                # pre-warm the exp table set while ACT is idle-ish
                warm = p1.tile([P, 1], F32, tag="warm")
                nc.scalar.activation(warm[:], psK[0][:, 0:1], AF.Exp, scale=1.0)
                for jr in range(NQC):
                    nc.scalar.copy(
                        out=vT[:, jr * QCH : (jr + 1) * QCH], in_=psV[jr][:]
                    )
                # rope kT chunk by chunk (perm matmul reuses freed V banks)
                for jr in range(NQC):
                    cs = slice(jr * QCH, (jr + 1) * QCH)
                    rot = psP.tile([P, QCH], F32, tag=f"pv{jr}",
                                   name=f"pv{jr}")
                    nc.tensor.matmul(rot[:], perm_t[:], kT[:, cs],
                                     start=True, stop=True)
                    rope_chunk(kT[:, cs], rot, cs)
                nc.sync.dma_start_transpose(vK[:], vT[:])

                # Q projections per head, ping-pong PSUM + rope per chunk
                for hh in range(NH):
                    for jr in range(NQC):
                        cs = slice(jr * QCH, (jr + 1) * QCH)
                        ps = psP.tile([P, QCH], F32, tag=f"pk{jr % 2}",
                                      name=f"pk{jr % 2}")
                        for kd in range(NKD):
                            nc.tensor.matmul(
                                ps[:],
                                wqt[:, kd, hh * P : (hh + 1) * P],
                                xf[:, kd, jr * QCH : (jr + 1) * QCH],
                                start=(kd == 0), stop=(kd == NKD - 1),
                            )
                        dst = qT[:, hh, cs]
                        nc.scalar.copy(out=dst, in_=ps[:])
                        rot = psP.tile([P, QCH], F32, tag=f"pk{2 + jr % 2}",
                                       name=f"pk{2 + jr % 2}")
                        nc.tensor.matmul(rot[:], perm_t[:], dst,
                                         start=True, stop=True)
                        rope_chunk(dst, rot, cs)

            # wo prefetched during phase 1 tail / attention start
            p3w_cm = tc.tile_pool(name="p3w", bufs=1)
            p3w = p3w_cm.__enter__()
            wo_t = p3w.tile([P, NH, D], F16)
            wo_vv = wo_d[:].rearrange("p (a o) -> p a o", a=NH)
            dma(wo_t[:, 0:2, :], wo_vv[:, 0:2, :])
            dma(wo_t[:, 2:4, :], wo_vv[:, 2:4, :])

            # ======== Phase 2: fused attention + output projection =======
            # PSUM budget (8 banks): s0,s1 = 2+2, ops = 1, dps = 1, po = 2.
            with tc.tile_pool(name="p2", bufs=1) as p2, \
                 tc.tile_pool(name="psF", bufs=1, space="PSUM") as psF:

                # pending output-projection emissions, drained into the
                # exp-latency gaps of the attention stream
                jobs = []
                njobs = NQC * (D // P)
                nemitted = [0]

                def drain(n):
                    for _ in range(n):
                        if not jobs:
                            return
                        jobs.pop(0)()

                def make_job(oc, jq):
                    def job():
                        nemitted[0] += 1
                        last = nemitted[0] > njobs - 3
                        po = psF.tile([P, QCH], F32, tag=f"po{oc % 2}",
                                      name=f"po{oc % 2}")
                        for a in range(NH):
                            nc.tensor.matmul(
                                po[:],
                                wo_t[:, a, oc * P : (oc + 1) * P],
                                attnT[:, a, jq * QCH : (jq + 1) * QCH],
                                start=(a == 0), stop=(a == NH - 1),
                            )
                        ot = p2.tile([P, QCH], F16, tag="ot", bufs=6)
                        if oc % 2 == 0:
                            nc.scalar.copy(out=ot[:], in_=po[:])
                        else:
                            nc.vector.tensor_copy(out=ot[:], in_=po[:])
                        dst = out_d[oc * P : (oc + 1) * P,
                                    jq * QCH : (jq + 1) * QCH]
                        if last:
                            # split the tail stores across DMA queues
                            for q4 in range(4):
                                dma(dst[q4 * 32 : (q4 + 1) * 32, :],
                                    ot[q4 * 32 : (q4 + 1) * 32, :])
                        else:
                            dma(dst, ot[:])
                    return job

                # jq1 first: its leading pairs are off-diagonal, so the
                # s tiles' first-ever writes are full-width (no stale
                # columns ever feed downstream), and by the time the
                # tiny DVE-bound jq0 block runs there are output-
                # projection jobs available to keep PE busy.
                for jq in (1, 0, 2, 3):
                    for h in range(NH):
                        nkc = 4 * (jq + 1)
                        npair = nkc // 2
                        qs = qT[:, h, jq * QCH : (jq + 1) * QCH]
                        ops = psF.tile([P, QCH], F32, tag="ops", name="ops")
                        pAcc = p2.tile([P, 2 * QCH], F16,
                                       tag="pAcc", bufs=2, name="pAcc")

                        # query offset of the live range for key chunk kc
                        # (0 off the diagonal)
                        def qoff(kc):
                            return max(0, KCH * (kc - 4 * jq))

                        def emit_qk(ip):
                            kc0 = 2 * ip
                            sps = psF.tile(
                                [P, 2 * QCH], F32, tag=f"s{ip % 2}",
                                name=f"sps{ip % 2}",
                            )
                            for k2 in range(2):
                                off = qoff(kc0 + k2)
                                nc.tensor.matmul(
                                    sps[:, k2 * QCH + off : (k2 + 1) * QCH],
                                    kT[:, (kc0 + k2) * P : (kc0 + k2 + 1) * P],
                                    qs[:, off:],
                                    start=True,
                                    stop=True,
                                )
                            return sps

                        sps_cur = emit_qk(0)
                        for ip in range(npair):
                            kc0 = 2 * ip
                            pT = p2.tile([P, 2 * QCH], F16, tag="pT", bufs=6)
                            nc.scalar.activation(
                                pT[:], sps_cur[:], AF.Exp, scale=SCALE
                            )
                            diag = kc0 >= 4 * jq
                            first_pair_init = ip == 0
                            if diag and first_pair_init:
                                # jq0 pair0: full-width mask (also zeroes
                                # dead/stale columns) since the masked tile
                                # is about to initialize pAcc via a full-
                                # width copy
                                nc.vector.tensor_tensor(
                                    pT[:], pT[:],
                                    mask_t[:, kc0 : kc0 + 2, :], OP.mult,
                                )
                            elif diag:
                                # mask only the 128-wide diagonal slivers
                                for k2 in range(2):
                                    off = qoff(kc0 + k2)
                                    sl = slice(k2 * QCH + off,
                                               k2 * QCH + off + KCH)
                                    nc.vector.tensor_tensor(
                                        pT[:, sl], pT[:, sl], tri_t[:],
                                        OP.mult,
                                    )
                            if ip + 1 < npair:
                                sps_cur = emit_qk(ip + 1)
                            for k2 in range(2):
                                kc = kc0 + k2
                                off = qoff(kc)
                                nc.tensor.matmul(
                                    ops[:, off:],
                                    vK[:, kc, :],
                                    pT[:, k2 * QCH + off : (k2 + 1) * QCH],
                                    start=(kc == 0),
                                    stop=(kc == nkc - 1),
                                )
                            drain(2 if jq == 0 else 1)
                            if first_pair_init:
                                nc.vector.tensor_copy(out=pAcc[:], in_=pT[:])
                            elif qoff(kc0) == 0 and qoff(kc0 + 1) == 0:
                                nc.vector.tensor_tensor(
                                    pAcc[:], pAcc[:], pT[:], OP.add
                                )
                            else:
                                for k2 in range(2):
                                    off = qoff(kc0 + k2)
                                    sl = slice(k2 * QCH + off, (k2 + 1) * QCH)
                                    nc.vector.tensor_tensor(
                                        pAcc[:, sl], pAcc[:, sl], pT[:, sl],
                                        OP.add,
                                    )
                        # cross-partition colsum of pAcc -> denominator
                        dps = psF.tile([P, QCH], F32, tag="dps", name="dps")
                        nc.tensor.matmul(
                            dps[:], ones_t[:], pAcc[:, 0:QCH],
                            start=True, stop=False,
                        )
                        nc.tensor.matmul(
                            dps[:], ones_t[:], pAcc[:, QCH : 2 * QCH],
                            start=False, stop=True,
                        )
                        # queue PE runway before the DVE recip/mult chain
                        drain(3)
                        dib = p2.tile([P, QCH], F32, tag="dib", bufs=2)
                        nc.vector.reciprocal_approx_fast(dib[:], dps[:])
                        nc.vector.tensor_tensor(
                            attnT[:, h, jq * QCH : (jq + 1) * QCH],
                            ops[:],
                            dib[:],
                            OP.mult,
                        )
                    for oc in range(D // P):
                        jobs.append(make_job(oc, jq))
                drain(len(jobs))
            p3w_cm.__exit__(None, None, None)

    nc.finalize()
    return nc


_NC = None


def _get_nc():
    global _NC
    if _NC is None:
        _NC = build_nc()
    return _NC


def _pack_pm(w):
    """[K, C] f32 -> [128, (K//128)*C] f16 partition-major pack:
    out[p, kd*C + c] = w[kd*128 + p, c]"""
    K, C = w.shape
    kd = K // P
    return np.ascontiguousarray(
        np.asarray(w, dtype=np.float16).reshape(kd, P, C).transpose(1, 0, 2)
    ).reshape(P, kd * C)


def make_in_maps(x, wq, wk, wv, wo):
    x = np.asarray(x, dtype=np.float32)
    in_maps = []
    for c in range(8):
        b, g = c // 4, c % 4
        in_maps.append(
            {
                "xT": np.ascontiguousarray(x[b].T).astype(np.float16),
                "wq": _pack_pm(wq[:, QW * g : QW * (g + 1)]),
                "wk": _pack_pm(wk[:, P * g : P * (g + 1)]),
                "wv": _pack_pm(wv[:, P * g : P * (g + 1)]),
                "wo": _pack_pm(wo[QW * g : QW * (g + 1), :]),
            }
        )
    return in_maps


def kernel(x, wq, wk, wv, wo):
    nc = _get_nc()
    in_maps = make_in_maps(x, wq, wk, wv, wo)
    res = run_bass_kernel_spmd(nc, in_maps, list(range(8)))
    parts = [res.results[c]["outT"].astype(np.float32) for c in range(8)]
    out = np.stack(
        [
            (parts[0] + parts[1] + parts[2] + parts[3]).T,
            (parts[4] + parts[5] + parts[6] + parts[7]).T,
        ]
    ).astype(np.float32)
    return out


# revision 19
# speedup vs baseline: 1.0209x; 1.0209x over previous
"""Causal GQA self-attention with RoPE for TRN2, 8 NeuronCores.

Problem: B=2, S=2048, D=2048, H=16 q-heads, KV=4 kv-heads, HD=128.

Sharding: core c = (batch b = c//4, kv-group g = c%4). Each core computes
q-heads 4g..4g+3 and kv-head g for batch b in the transposed (S^T) layout,
then a partial output projection; host sums the 4 partials per batch.

v4 notes (PE is the bottleneck; keep it saturated end to end):
  - weights pre-packed on host into SBUF partition-major layout (4-16KB
    DMA descriptors); DMA dispatch costs ~650ns each on a queue, so
    loads are dual-dispatched from the sync AND gpsimd queues, ordered
    by need (wk, wv, x chunks, ...). K/V projections run kd-outer,
    chasing the x stream.
  - RoPE rotate-half is a PE permutation matmul per [128,512] chunk
    (engines cannot cross partitions; the old SBUF->SBUF swap DMA had
    ~7us latency and stalled attention start), then 3 narrow DVE ops.
  - softmax denominator via DVE accumulation of exp tiles + one
    ones-matmul colsum pair per (head, query block).
  - causal masking: only the 128-wide diagonal sliver of each diagonal
    key chunk is mask-multiplied (every diagonal chunk uses the same
    triangular [128,128] mask); QK/PV matmuls and the DVE accumulation
    are narrowed to the live query range.
  - attention runs jq-outer / h-inner; output-projection matmuls are
    drained into the exp-latency gaps of the attention stream.
  - output stored f16; host sums partials in f32.
"""
import sys

sys.path.insert(0, "/opt/trn_rl_repo")

import numpy as np

import concourse.tile as tile
from concourse import bacc, mybir
from concourse.bass_utils import run_bass_kernel_spmd

F32 = mybir.dt.float32
F16 = mybir.dt.float16
AF = mybir.ActivationFunctionType
OP = mybir.AluOpType

P = 128          # partitions / head dim
S = 2048         # sequence length
D = 2048         # model dim
NH = 4           # q heads per core
QW = NH * P      # q projection width per core (512)
NKD = D // P     # contraction chunks (16)
QCH = 512        # query chunk (free dim of attention matmuls)
NQC = S // QCH   # 4
KCH = P          # key chunk (128, on partitions)
NKC = S // KCH   # 16
SCALE = float(P) ** -0.5


def _host_constants():
    inv = 1.0 / (10000.0 ** (np.arange(0, P, 2, dtype=np.float64) / P))  # [64]
    pos = np.arange(S, dtype=np.float64)
    freqs = pos[:, None] * inv[None, :]                  # [S, 64]
    emb = np.concatenate([freqs, freqs], axis=-1)        # [S, 128]
    cosT = np.cos(emb).T.astype(np.float16).copy()       # [128, S]
    sinT = np.sin(emb).T.astype(np.float16)
    sinT[: P // 2] *= np.float16(-1.0)                   # fold rotate_half sign
    sinT = sinT.copy()
    # triangular mask for the 128-wide diagonal sliver: m[p, q] = q >= p
    tri = (np.arange(P)[None, :] >= np.arange(P)[:, None]).astype(np.float16)
    # full step masks for the jq0/h0 full-width path: mask[p,j,q] = q >= p+128j
    q = np.arange(QCH)[None, None, :]
    p = np.arange(P)[:, None, None]
    j = np.arange(4)[None, :, None]
    masks = (q >= p + KCH * j).astype(np.float16)        # [128, 4, 512]
    ones = np.ones((P, P), dtype=np.float16)
    # rotate-half permutation: out[m] = in[(m+64) % 128]
    perm = np.zeros((P, P), dtype=np.float16)
    perm[(np.arange(P) + P // 2) % P, np.arange(P)] = 1.0
    return cosT, sinT, tri, masks, ones, perm


def build_nc():
    cosT_np, sinT_np, tri_np, masks_np, ones_np, perm_np = _host_constants()

    nc = bacc.Bacc(None)
    # weights arrive pre-packed as [128, ...] partition-major arrays
    xT_d = nc.dram_tensor("xT", [D, S], F16, kind="ExternalInput")
    wq_d = nc.dram_tensor("wq", [P, NKD * QW], F16, kind="ExternalInput")
    wk_d = nc.dram_tensor("wk", [P, NKD * P], F16, kind="ExternalInput")
    wv_d = nc.dram_tensor("wv", [P, NKD * P], F16, kind="ExternalInput")
    wo_d = nc.dram_tensor("wo", [P, NH * D], F16, kind="ExternalInput")
    out_d = nc.dram_tensor("outT", [D, S], F16, kind="ExternalOutput")

    cos_d = nc.inline_tensor(cosT_np, name="cosT")
    sin_d = nc.inline_tensor(sinT_np, name="sinT")
    tri_d = nc.inline_tensor(tri_np, name="trim")
    mask_d = nc.inline_tensor(masks_np, name="masks")
    ones_d = nc.inline_tensor(ones_np, name="onesm")
    perm_d = nc.inline_tensor(perm_np, name="permm")

    xT_v = xT_d[:].rearrange("(kd p) s -> p kd s", p=P)

    # alternate DMA dispatch between the sync and gpsimd queues
    dq = [0]

    def dma(out, in_):
        eng = nc.sync if dq[0] % 2 == 0 else nc.gpsimd
        dq[0] += 1
        eng.dma_start(out, in_)

    with tile.TileContext(nc) as tc:
        with tc.tile_pool(name="persist", bufs=1) as pp:
            qT = pp.tile([P, NH, S], F16)        # q^T; attention overwrites
            kT = pp.tile([P, S], F16)
            vT = pp.tile([P, S], F16)
            vK = pp.tile([P, NKC, P], F16)       # V as (kpos, kchunk, hd)
            tri_t = pp.tile([P, P], F16)
            mask_t = pp.tile([P, 4, QCH], F16)
            ones_t = pp.tile([P, P], F16)        # all-ones for colsum
            perm_t = pp.tile([P, P], F16)

            # attention output overwrites qT in place: slice (h, jq-chunk) is
            # written only after every read of that same slice is done.
            attnT = qT

            # ======== Phase 1: QKV projections + RoPE ====================
            with tc.tile_pool(name="xp", bufs=1) as xp, \
                 tc.tile_pool(name="wp", bufs=1) as wp, \
                 tc.tile_pool(name="p1", bufs=2) as p1, \
                 tc.tile_pool(name="p1c", bufs=1) as p1c, \
                 tc.tile_pool(name="psP", bufs=1, space="PSUM") as psP:
                # dispatch order == need order: wk, wv, x chunks, the rest
                wkt = wp.tile([P, NKD, P], F16)
                wvt = wp.tile([P, NKD, P], F16)
                nc.sync.dma_start(
                    wkt[:], wk_d[:].rearrange("p (kd c) -> p kd c", c=P))
                nc.gpsimd.dma_start(
                    wvt[:], wv_d[:].rearrange("p (kd c) -> p kd c", c=P))
                xf = xp.tile([P, NKD, S], F16)
                for kd in range(NKD):
                    dma(xf[:, kd, :], xT_v[:, kd, :])
                cos_t = p1c.tile([P, S], F16)
                sin_t = p1c.tile([P, S], F16)
                dma(cos_t[:], cos_d[:])
                dma(sin_t[:], sin_d[:])
                dma(perm_t[:], perm_d[:])
                wqt = wp.tile([P, NKD, QW], F16)
                wq_vv = wq_d[:].rearrange("p (kd c) -> p kd c", c=QW)
                for k4 in range(4):
                    dma(wqt[:, 4 * k4 : 4 * (k4 + 1), :],
                        wq_vv[:, 4 * k4 : 4 * (k4 + 1), :])
                dma(tri_t[:], tri_d[:])
                dma(mask_t[:], mask_d[:])
                dma(ones_t[:], ones_d[:])

                def rope_chunk(dst, rot_ps, cslice):
                    # dst = dst*cos + perm(dst)*sin ; rot_ps holds perm(dst)
                    tmpv = p1.tile([P, QCH], F16, tag="ropet", bufs=3)
                    nc.vector.tensor_tensor(
                        tmpv[:], rot_ps[:], sin_t[:, cslice], OP.mult)
                    nc.vector.tensor_tensor(
                        dst, dst, cos_t[:, cslice], OP.mult)
                    nc.vector.tensor_tensor(dst, dst, tmpv[:], OP.add)

                # K and V projections, kd-outer, 8 PSUM accumulators
                psK = [psP.tile([P, QCH], F32, tag=f"pk{jr}", name=f"pk{jr}")
                       for jr in range(NQC)]
                psV = [psP.tile([P, QCH], F32, tag=f"pv{jr}", name=f"pv{jr}")
                       for jr in range(NQC)]
                for kd in range(NKD):
                    for jr in range(NQC):
                        nc.tensor.matmul(
                            psK[jr][:], wkt[:, kd, :],
                            xf[:, kd, jr * QCH : (jr + 1) * QCH],
                            start=(kd == 0), stop=(kd == NKD - 1),
                        )
                    for jr in range(NQC):
                        nc.tensor.matmul(
                            psV[jr][:], wvt[:, kd, :],
                            xf[:, kd, jr * QCH : (jr + 1) * QCH],
                            start=(kd == 0), stop=(kd == NKD - 1),
                        )
                for jr in range(NQC):
                    nc.scalar.copy(
                        out=kT[:, jr * QCH : (jr + 1) * QCH], in_=psK[jr][:]
                    )
                # pre-warm the exp table set while ACT is idle-ish
                warm = p1.tile([P, 1], F32, tag="warm")
                nc.scalar.activation(warm[:], psK[0][:, 0:1], AF.Exp, scale=1.0)
                for jr in range(NQC):
                    nc.scalar.copy(
                        out=vT[:, jr * QCH : (jr + 1) * QCH], in_=psV[jr][:]
                    )
                # rope kT chunk by chunk (perm matmul reuses freed V banks)
                for jr in range(NQC):
                    cs = slice(jr * QCH, (jr + 1) * QCH)
                    rot = psP.tile([P, QCH], F32, tag=f"pv{jr}",
                                   name=f"pv{jr}")
                    nc.tensor.matmul(rot[:], perm_t[:], kT[:, cs],
                                     start=True, stop=True)
                    rope_chunk(kT[:, cs], rot, cs)
                nc.sync.dma_start_transpose(vK[:], vT[:])

                # Q projections per head, ping-pong PSUM + rope per chunk
                for hh in range(NH):
                    for jr in range(NQC):
                        cs = slice(jr * QCH, (jr + 1) * QCH)
                        ps = psP.tile([P, QCH], F32, tag=f"pk{jr % 2}",
                                      name=f"pk{jr % 2}")
                        for kd in range(NKD):
                            nc.tensor.matmul(
                                ps[:],
                                wqt[:, kd, hh * P : (hh + 1) * P],
                                xf[:, kd, jr * QCH : (jr + 1) * QCH],
                                start=(kd == 0), stop=(kd == NKD - 1),
                            )
                        dst = qT[:, hh, cs]
                        nc.scalar.copy(out=dst, in_=ps[:])
                        rot = psP.tile([P, QCH], F32, tag=f"pk{2 + jr % 2}",
                                       name=f"pk{2 + jr % 2}")
                        nc.tensor.matmul(rot[:], perm_t[:], dst,
                                         start=True, stop=True)
                        rope_chunk(dst, rot, cs)

            # wo prefetched during phase 1 tail / attention start
            p3w_cm = tc.tile_pool(name="p3w", bufs=1)
            p3w = p3w_cm.__enter__()
            wo_t = p3w.tile([P, NH, D], F16)
            wo_vv = wo_d[:].rearrange("p (a o) -> p a o", a=NH)
            dma(wo_t[:, 0:2, :], wo_vv[:, 0:2, :])
            dma(wo_t[:, 2:4, :], wo_vv[:, 2:4, :])

            # ======== Phase 2: fused attention + output projection =======
            # PSUM budget (8 banks): s0,s1 = 2+2, ops = 1, dps = 1, po = 2.
            with tc.tile_pool(name="p2", bufs=1) as p2, \
                 tc.tile_pool(name="psF", bufs=1, space="PSUM") as psF:

                # pending output-projection emissions, drained into the
                # exp-latency gaps of the attention stream
                jobs = []
                njobs = NQC * (D // P)
                nemitted = [0]

                def drain(n):
                    for _ in range(n):
                        if not jobs:
                            return
                        jobs.pop(0)()

                def make_job(oc, jq):
                    def job():
                        nemitted[0] += 1
                        last = nemitted[0] > njobs - 3
                        po = psF.tile([P, QCH], F32, tag="po", bufs=3,
                                      name="po")
                        for a in range(NH):
                            nc.tensor.matmul(
                                po[:],
                                wo_t[:, a, oc * P : (oc + 1) * P],
                                attnT[:, a, jq * QCH : (jq + 1) * QCH],
                                start=(a == 0), stop=(a == NH - 1),
                            )
                        ot = p2.tile([P, QCH], F16, tag="ot", bufs=6)
                        if oc % 2 == 0:
                            nc.scalar.copy(out=ot[:], in_=po[:])
                        else:
                            nc.vector.tensor_copy(out=ot[:], in_=po[:])
                        dst = out_d[oc * P : (oc + 1) * P,
                                    jq * QCH : (jq + 1) * QCH]
                        if last:
                            # split the tail stores across DMA queues
                            for q4 in range(4):
                                dma(dst[q4 * 32 : (q4 + 1) * 32, :],
                                    ot[q4 * 32 : (q4 + 1) * 32, :])
                        else:
                            dma(dst, ot[:])
                    return job

                # jq1 first: its leading pairs are off-diagonal, so the
                # s tiles' first-ever writes are full-width (no stale
                # columns ever feed downstream), and by the time the
                # tiny DVE-bound jq0 block runs there are output-
                # projection jobs available to keep PE busy.
                for jq in (1, 0, 2, 3):
                    for h in range(NH):
                        nkc = 4 * (jq + 1)
                        npair = nkc // 2
                        qs = qT[:, h, jq * QCH : (jq + 1) * QCH]
                        ops = psF.tile([P, QCH], F32, tag="ops", name="ops")
                        pAcc = p2.tile([P, 2 * QCH], F16,
                                       tag="pAcc", bufs=2, name="pAcc")

                        # query offset of the live range for key chunk kc
                        # (0 off the diagonal)
                        def qoff(kc):
                            return max(0, KCH * (kc - 4 * jq))

                        def emit_qk(ip):
                            kc0 = 2 * ip
                            sps = psF.tile(
                                [P, 2 * QCH], F32, tag=f"s{ip % 2}",
                                name=f"sps{ip % 2}",
                            )
                            for k2 in range(2):
                                off = qoff(kc0 + k2)
                                nc.tensor.matmul(
                                    sps[:, k2 * QCH + off : (k2 + 1) * QCH],
                                    kT[:, (kc0 + k2) * P : (kc0 + k2 + 1) * P],
                                    qs[:, off:],
                                    start=True,
                                    stop=True,
                                )
                            return sps

                        sps_cur = emit_qk(0)
                        for ip in range(npair):
                            kc0 = 2 * ip
                            pT = p2.tile([P, 2 * QCH], F16, tag="pT", bufs=6)
                            nc.scalar.activation(
                                pT[:], sps_cur[:], AF.Exp, scale=SCALE
                            )
                            diag = kc0 >= 4 * jq
                            first_pair_init = ip == 0
                            if diag and first_pair_init:
                                # jq0 pair0: full-width mask (also zeroes
                                # dead/stale columns) since the masked tile
                                # is about to initialize pAcc via a full-
                                # width copy
                                nc.vector.tensor_tensor(
                                    pT[:], pT[:],
                                    mask_t[:, kc0 : kc0 + 2, :], OP.mult,
                                )
                            elif diag:
                                # mask only the 128-wide diagonal slivers
                                for k2 in range(2):
                                    off = qoff(kc0 + k2)
                                    sl = slice(k2 * QCH + off,
                                               k2 * QCH + off + KCH)
                                    nc.vector.tensor_tensor(
                                        pT[:, sl], pT[:, sl], tri_t[:],
                                        OP.mult,
                                    )
                            if ip + 1 < npair:
                                sps_cur = emit_qk(ip + 1)
                            for k2 in range(2):
                                kc = kc0 + k2
                                off = qoff(kc)
                                nc.tensor.matmul(
                                    ops[:, off:],
                                    vK[:, kc, :],
                                    pT[:, k2 * QCH + off : (k2 + 1) * QCH],
                                    start=(kc == 0),
                                    stop=(kc == nkc - 1),
                                )
                            drain(2 if jq == 0 else 1)
                            if first_pair_init:
                                nc.vector.tensor_copy(out=pAcc[:], in_=pT[:])
                            elif qoff(kc0) == 0 and qoff(kc0 + 1) == 0:
                                nc.vector.tensor_tensor(
                                    pAcc[:], pAcc[:], pT[:], OP.add
                                )
                            else:
                                for k2 in range(2):
                                    off = qoff(kc0 + k2)
                                    sl = slice(k2 * QCH + off, (k2 + 1) * QCH)
                                    nc.vector.tensor_tensor(
                                        pAcc[:, sl], pAcc[:, sl], pT[:, sl],
                                        OP.add,
                                    )
                        # cross-partition colsum of pAcc -> denominator
                        # (dps shares the po ring: 8th PSUM bank deepens
                        # the output-projection pipeline instead)
                        dps = psF.tile([P, QCH], F32, tag="po", bufs=3,
                                       name="po")
                        if jq >= 2:
                            # DVE has slack here: pre-add the halves so
                            # one colsum matmul suffices
                            pAccH = p2.tile([P, QCH], F16, tag="pAccH",
                                            bufs=2)
                            nc.vector.tensor_tensor(
                                pAccH[:], pAcc[:, 0:QCH],
                                pAcc[:, QCH : 2 * QCH], OP.add,
                            )
                            nc.tensor.matmul(
                                dps[:], ones_t[:], pAccH[:],
                                start=True, stop=True,
                            )
                        else:
                            nc.tensor.matmul(
                                dps[:], ones_t[:], pAcc[:, 0:QCH],
                                start=True, stop=False,
                            )
                            nc.tensor.matmul(
                                dps[:], ones_t[:], pAcc[:, QCH : 2 * QCH],
                                start=False, stop=True,
                            )
                        # queue PE runway before the DVE recip/mult chain
                        drain(3)
                        dib = p2.tile([P, QCH], F32, tag="dib", bufs=2)
                        nc.vector.reciprocal_approx_fast(dib[:], dps[:])
                        nc.vector.tensor_tensor(
                            attnT[:, h, jq * QCH : (jq + 1) * QCH],
                            ops[:],
                            dib[:],
                            OP.mult,
                        )
                    for oc in range(D // P):
                        jobs.append(make_job(oc, jq))
                drain(len(jobs))
            p3w_cm.__exit__(None, None, None)

    nc.finalize()
    return nc


_NC = None


def _get_nc():
    global _NC
    if _NC is None:
        _NC = build_nc()
    return _NC


def _pack_pm(w):
    """[K, C] f32 -> [128, (K//128)*C] f16 partition-major pack:
    out[p, kd*C + c] = w[kd*128 + p, c]"""
    K, C = w.shape
    kd = K // P
    return np.ascontiguousarray(
        np.asarray(w, dtype=np.float16).reshape(kd, P, C).transpose(1, 0, 2)
    ).reshape(P, kd * C)


def make_in_maps(x, wq, wk, wv, wo):
    x = np.asarray(x, dtype=np.float32)
    in_maps = []
    for c in range(8):
        b, g = c // 4, c % 4
        in_maps.append(
            {
                "xT": np.ascontiguousarray(x[b].T).astype(np.float16),
                "wq": _pack_pm(wq[:, QW * g : QW * (g + 1)]),
                "wk": _pack_pm(wk[:, P * g : P * (g + 1)]),
                "wv": _pack_pm(wv[:, P * g : P * (g + 1)]),
                "wo": _pack_pm(wo[QW * g : QW * (g + 1), :]),
            }
        )
    return in_maps


def kernel(x, wq, wk, wv, wo):
    nc = _get_nc()
    in_maps = make_in_maps(x, wq, wk, wv, wo)
    res = run_bass_kernel_spmd(nc, in_maps, list(range(8)))
    parts = [res.results[c]["outT"].astype(np.float32) for c in range(8)]
    out = np.stack(
        [
            (parts[0] + parts[1] + parts[2] + parts[3]).T,
            (parts[4] + parts[5] + parts[6] + parts[7]).T,
        ]
    ).astype(np.float32)
    return out
